# revision 3
# baseline (speedup 1.0000x reference)
"""EMAStats segment-reduce kernel for 8 Trainium2 NeuronCores (Bass/Tile).

Problem: given logits [B, K], target [B], running (mean, var, count) [K]:
  own[i]     = logits[i, target[i]]
  per class c: n_c = #{i: t_i=c}, s_c = sum own, q_c = sum own^2
  batch_mean = s/n, batch_var = q/n - batch_mean^2
  EMA update with decay 0.1 (first update uses batch stats); classes with
  n_c = 0 keep their buffers.

v2 strategy (data-parallel over B, 8 cores, BS = 16384 rows/core):
  1. idx build without slow strided DMAs: load target contiguously as
     tnat[128,128], PE-transpose to tl[p,q] = tgt[128q+p], compute
     idxf = (128*(q%8)+p)*32 | (tl>>6) on DVE, then distribute into the
     16-partition-wrapped gather index layout with 8 one-hot PE matmuls
     (out[P,q] = idxf[16*hi + P%16, q]) + 8 psum->int16 copies. This also
     produces the 8-fold partition replication dma_gather requires.
  2. 16 dma_gathers (SWDGE, 4 queues) fetch each row's 256-byte block
     holding column target[i] (4 MiB instead of the 128 MiB shard).
  3. Extraction: own = reduce(G * onehot64(lowb)) per 32-column quarter.
  4. Histogram via PE in bf16: class k = A*16 + g; per token column a
     one-hot over A (bf16) and 64 masked-stat columns (16 g-groups x
     {count, hi, lo, sq}) accumulate psum[A, 64] over 128 matmuls.
     Mean uses an hi/lo bf16 split of own (s = sum hi + sum lo) so the
     bf16 matmul loses no mean precision; sq tolerates bf16 rounding.
     Matmuls are emitted per quarter so PE overlaps extraction.
  5. All-reduce of the [128, 48] partials WITHOUT the ncfw collective
     (~40us floor): XOR all-gather with remote_dma_broadcast inside a
     tile_critical section - each core sends its partials to peer
     (me^d)'s slot d (slot d of core r therefore holds core (r^d)'s
     tile), waits remote_sem >= 14, and sums the 8 slots locally.
  6. EMA update applied redundantly on every core (K = 2048 is tiny).
"""

import numpy as np

import concourse.bacc as bacc
import concourse.bass as bass
import concourse.mybir as mybir
import concourse.tile as tile
from concourse.bass_utils import run_bass_kernel_spmd

B, K = 131072, 2048
NCORES = 8
BS = B // NCORES  # 16384 rows per core
P = 128
NG = 16  # g-groups (class & 15)
NST = 4  # stats per group: cnt, hi, lo, sq
NSUB = 16  # sub-shards for int16 gather indices
SUBR = BS // NSUB  # 1024 rows per sub-shard
BLK = 64  # f32 elements per gathered block (256 bytes)
EMA_DECAY = 0.1
EPS = 1e-12

F32 = mybir.dt.float32
BF16 = mybir.dt.bfloat16
I32 = mybir.dt.int32
I16 = mybir.dt.int16

OP = mybir.AluOpType


def build_program() -> bass.Bass:
    nc = bacc.Bacc(
        trn_type="TRN2", num_devices=NCORES, debug=False, num_swdge_queues=4
    )

    lg = nc.dram_tensor("logits", [BS * K, 1], F32, kind="ExternalInput")
    tgt = nc.dram_tensor("target", [BS], I32, kind="ExternalInput")
    mean_in = nc.dram_tensor("mean", [K], F32, kind="ExternalInput")
    var_in = nc.dram_tensor("var", [K], F32, kind="ExternalInput")
    cnt_in = nc.dram_tensor("count", [K], I32, kind="ExternalInput")

    new_mean = nc.dram_tensor("new_mean", [K], F32, kind="ExternalOutput")
    new_var = nc.dram_tensor("new_var", [K], F32, kind="ExternalOutput")
    new_count = nc.dram_tensor("new_count", [K], I32, kind="ExternalOutput")

    # --- inline constants --------------------------------------------------
    ident_c = nc.inline_tensor(np.eye(P, dtype=np.float32), name="ident_c")
    iota_row = np.broadcast_to(np.arange(P, dtype=np.int32), (P, P)).copy()
    iota_t_c = nc.inline_tensor(iota_row, name="iota_t_c")
    iota64_c = nc.inline_tensor(
        np.broadcast_to(np.arange(BLK, dtype=np.int32), (P, BLK)).copy(),
        name="iota64_c",
    )
    # aff[p, q] = (128*(q%8) + p) * 32  (row-within-gather * 32)
    pp, qq = np.mgrid[0:P, 0:P]
    aff_np = ((128 * (qq % 8) + pp) * 32).astype(np.int32)
    aff_c = nc.inline_tensor(aff_np, name="aff_c")
    # shuf[p, 128*hi + Pd] = 1 if p == 16*hi + (Pd % 16)
    shuf_np = np.zeros((P, 8 * P), dtype=np.float32)
    for hi in range(8):
        for Pd in range(P):
            shuf_np[16 * hi + (Pd % 16), 128 * hi + Pd] = 1.0
    shuf_c = nc.inline_tensor(shuf_np, name="shuf_c")

    with tile.TileContext(nc) as tc:
        with (
            tc.tile_pool(name="sb", bufs=1) as sb,
            tc.tile_pool(name="oh", bufs=16) as ohp,
            tc.tile_pool(name="ex", bufs=2) as exp_,
            tc.tile_pool(name="ps", bufs=1, space="PSUM") as ps,
            tc.tile_pool(name="psh", bufs=2, space="PSUM") as psh_pool,
        ):
            # --- load target + constants ---------------------------------
            tnat = sb.tile([P, P], I32)
            nc.sync.dma_start(out=tnat[:], in_=tgt[:].rearrange("(p f) -> p f", p=P))
            ident = sb.tile([P, P], F32)
            nc.scalar.dma_start(out=ident[:], in_=ident_c[:, :])
            aff_t = sb.tile([P, P], I32)
            nc.scalar.dma_start(out=aff_t[:], in_=aff_c[:, :])
            shuf_t = sb.tile([P, 8 * P], F32)
            nc.sync.dma_start(out=shuf_t[:], in_=shuf_c[:, :])
            iota64s = sb.tile([P, BLK], I32)
            nc.scalar.dma_start(out=iota64s[:], in_=iota64_c[:, :])
            iota_t = sb.tile([P, P], I32)
            nc.scalar.dma_start(out=iota_t[:], in_=iota_t_c[:, :])

            # --- TL[p, q] = target[q*128 + p] via PE transpose ------------
            tnatf = sb.tile([P, P], F32)
            nc.vector.tensor_copy(out=tnatf[:], in_=tnat[:])
            ptr = ps.tile([P, P], F32, name="ptr")
            nc.tensor.transpose(out=ptr[:], in_=tnatf[:], identity=ident[:])
            tl = sb.tile([P, P], I32)
            nc.vector.tensor_copy(out=tl[:], in_=ptr[:])

            # --- gather indices: idxf = (row-in-gather)*32 | (tgt >> 6) ---
            blk_t = sb.tile([P, P], I32)
            nc.vector.tensor_scalar(
                out=blk_t[:], in0=tl[:], scalar1=6, scalar2=None,
                op0=OP.arith_shift_right,
            )
            idxf = sb.tile([P, P], I32)
            nc.vector.tensor_tensor(
                out=idxf[:], in0=blk_t[:], in1=aff_t[:], op=OP.bitwise_or
            )
            idxff = sb.tile([P, P], F32)
            nc.vector.tensor_copy(out=idxff[:], in_=idxf[:])

            # distribute to the 16-wrapped layout (8 replicas) with PE:
            # idx16[Pd, 64*(q//8) + 8*(q%8) + hi] = idxf[16*hi + Pd%16, q]
            idx16 = sb.tile([P, BS // 16], I16)
            i16v = idx16[:].rearrange("p (G c h) -> p G c h", c=8, h=8)
            for hi in range(8):
                psh = psh_pool.tile([P, P], F32, name=f"psh_{hi}", tag="psh")
                nc.tensor.matmul(
                    out=psh[:],
                    lhsT=shuf_t[:, P * hi : P * (hi + 1)],
                    rhs=idxff[:],
                    start=True,
                    stop=True,
                )
                nc.vector.tensor_copy(
                    out=i16v[:, :, :, hi],
                    in_=psh[:].rearrange("p (G c) -> p G c", c=8),
                )

            # --- gathers: 16 sub-shards, 4 SWDGE queues -------------------
            lgb = lg[:].rearrange("(r e) x -> r (e x)", e=BLK)  # [BS*32, 64]
            g_t = sb.tile([P, P * BLK], F32)
            g3 = g_t[:].rearrange("p (q e) -> p q e", e=BLK)
            g3_slices = [g3[:, 8 * g : 8 * (g + 1), :] for g in range(NSUB)]
            for g in range(NSUB):
                nc.gpsimd.dma_gather(
                    g3_slices[g],
                    lgb[g * SUBR * (K // BLK) : (g + 1) * SUBR * (K // BLK), :],
                    idx16[:, (SUBR // 16) * g : (SUBR // 16) * (g + 1)],
                    SUBR,
                    SUBR,
                    elem_size=BLK,
                    queue_num=g % 4,
                )

            # --- class decomposition: k = A*16 + g ------------------------
            a_t = sb.tile([P, P], I32)
            b_t = sb.tile([P, P], I32)
            lowb = sb.tile([P, P], I32)
            nc.vector.tensor_scalar(
                out=a_t[:], in0=tl[:], scalar1=4, scalar2=None,
                op0=OP.arith_shift_right,
            )
            nc.vector.tensor_scalar(
                out=b_t[:], in0=tl[:], scalar1=15, scalar2=None,
                op0=OP.bitwise_and,
            )
            nc.vector.tensor_scalar(
                out=lowb[:], in0=tl[:], scalar1=BLK - 1, scalar2=None,
                op0=OP.bitwise_and,
            )

            # --- EMA inputs + first-mask (independent of partials) --------
            m_t = sb.tile([P, NG], F32)
            va_t = sb.tile([P, NG], F32)
            c_t = sb.tile([P, NG], I32)
            nc.sync.dma_start(out=m_t[:], in_=mean_in[:].rearrange("(p c) -> p c", p=P))
            nc.sync.dma_start(out=va_t[:], in_=var_in[:].rearrange("(p c) -> p c", p=P))
            nc.sync.dma_start(out=c_t[:], in_=cnt_in[:].rearrange("(p c) -> p c", p=P))
            cf_t = sb.tile([P, NG], F32)
            first_t = sb.tile([P, NG], mybir.dt.uint8)
            nc.vector.tensor_copy(out=cf_t[:], in_=c_t[:])
            nc.vector.tensor_scalar(
                out=first_t[:], in0=cf_t[:], scalar1=0.0, scalar2=None,
                op0=OP.is_equal,
            )

            # --- per-group count masks (bf16), built once -----------------
            vmall = sb.tile([P, P * NG * NST], BF16)
            vm4 = vmall[:].rearrange("p (c g s) -> p c g s", g=NG, s=NST)
            for g in range(NG):
                nc.vector.tensor_scalar(
                    out=vm4[:, :, g, 0], in0=b_t[:], scalar1=float(g),
                    scalar2=None, op0=OP.is_equal,
                )

            # --- extraction + histogram, one 32-column quarter at a time --
            oh8s = [None] * NSUB

            def build_oh8(j):
                oh8 = ohp.tile([P, 8 * P], BF16, name=f"oh8_{j}", tag="oh8")
                nc.vector.tensor_tensor(
                    out=oh8[:].rearrange("p (c a) -> p c a", a=P),
                    in0=a_t[:, 8 * j : 8 * (j + 1)][:, :, None].to_broadcast(
                        [P, 8, P]
                    ),
                    in1=iota_t[:, None, :].to_broadcast([P, 8, P]),
                    op=OP.is_equal,
                )
                oh8s[j] = oh8

            v = sb.tile([P, P], F32)
            hi_bf = sb.tile([P, P], BF16)
            hi_f = sb.tile([P, P], F32)
            lo_f = sb.tile([P, P], F32)
            sq_f = sb.tile([P, P], F32)
            pstats = ps.tile([P, NG * NST], F32)
            QT = P // 4
            for qt in range(4):
                cs = slice(QT * qt, QT * (qt + 1))
                for j in range(4 * qt, 4 * qt + 4):
                    build_oh8(j)
                ohq = exp_.tile([P, QT * BLK], F32, name=f"ohq_{qt}", tag="ohq")
                ohq3 = ohq[:].rearrange("p (q e) -> p q e", e=BLK)
                nc.vector.tensor_tensor(
                    out=ohq3[:],
                    in0=lowb[:, cs][:, :, None].to_broadcast([P, QT, BLK]),
                    in1=iota64s[:, None, :].to_broadcast([P, QT, BLK]),
                    op=OP.is_equal,
                )
                nc.vector.tensor_tensor(
                    out=ohq3[:], in0=g3[:, cs, :], in1=ohq3[:], op=OP.mult
                )
                nc.vector.tensor_reduce(
                    out=v[:, cs], in_=ohq3[:], axis=mybir.AxisListType.X, op=OP.add
                )
                # hi/lo bf16 split of own + squared values
                nc.vector.tensor_copy(out=hi_bf[:, cs], in_=v[:, cs])
                nc.vector.tensor_copy(out=hi_f[:, cs], in_=hi_bf[:, cs])
                nc.vector.tensor_tensor(
                    out=lo_f[:, cs], in0=v[:, cs], in1=hi_f[:, cs], op=OP.subtract
                )
                nc.vector.tensor_tensor(
                    out=sq_f[:, cs], in0=v[:, cs], in1=v[:, cs], op=OP.mult
                )
                for g in range(NG):
                    nc.vector.scalar_tensor_tensor(
                        out=vm4[:, cs, g, 1], in0=b_t[:, cs], scalar=float(g),
                        in1=hi_f[:, cs], op0=OP.is_equal, op1=OP.mult,
                    )
                    nc.vector.scalar_tensor_tensor(
                        out=vm4[:, cs, g, 2], in0=b_t[:, cs], scalar=float(g),
                        in1=lo_f[:, cs], op0=OP.is_equal, op1=OP.mult,
                    )
                    nc.vector.scalar_tensor_tensor(
                        out=vm4[:, cs, g, 3], in0=b_t[:, cs], scalar=float(g),
                        in1=sq_f[:, cs], op0=OP.is_equal, op1=OP.mult,
                    )
                # histogram matmuls for this quarter (bf16)
                for c in range(QT * qt, QT * (qt + 1)):
                    nc.tensor.matmul(
                        out=pstats[:],
                        lhsT=oh8s[c // 8][:, P * (c % 8) : P * (c % 8 + 1)],
                        rhs=vm4[:, c, :, :],
                        start=(c == 0),
                        stop=(c == P - 1),
                    )

            # --- local partials st[A, (stat, g)]; s = hi + lo -------------
            hsb = sb.tile([P, NG * NST], F32)
            nc.vector.tensor_copy(out=hsb[:], in_=pstats[:])
            hs = hsb[:].rearrange("p (g s) -> p g s", s=NST)
            st = sb.tile([P, 3 * NG], F32)
            st3 = st[:].rearrange("p (s g) -> p s g", s=3)
            nc.vector.tensor_copy(out=st3[:, 0, :], in_=hs[:, :, 0])
            nc.vector.tensor_tensor(
                out=st3[:, 1, :], in0=hs[:, :, 1], in1=hs[:, :, 2], op=OP.add
            )
            nc.vector.tensor_copy(out=st3[:, 2, :], in_=hs[:, :, 3])

            # --- XOR all-gather over the 8 cores (remote SBUF DMA) --------
            g8 = sb.tile([P, NCORES, 3 * NG], F32)
            gsum = sb.tile([P, 3 * NG], F32)
            rsem = nc.alloc_semaphore("ag_rsem")
            lsem = nc.alloc_semaphore("ag_lsem")
            psem = nc.alloc_semaphore("ag_psem")
            nc.vector.tensor_copy(out=g8[:, 0, :], in_=st[:])
            with tc.tile_critical(name="allgather"):
                for d in range(1, NCORES):
                    rdests = [(0, d) if k == d else None for k in range(NCORES)]
                    nc.gpsimd.remote_dma_broadcast(
                        out_ap=g8[:, d, :],
                        in_ap=st[:],
                        remote_sem=rsem,
                        local_sem=lsem,
                        rdests=rdests,
                        queue_num=0,
                    ).then_inc(psem, 1)
                nc.gpsimd.wait_ge(psem, NCORES - 1)
                nc.gpsimd.trigger_dma(count=NCORES - 1)
                nc.vector.wait_ge(rsem, 2 * (NCORES - 1))
                nc.vector.tensor_reduce(
                    out=gsum[:],
                    in_=g8[:].rearrange("p d w -> p w d"),
                    axis=mybir.AxisListType.X,
                    op=OP.add,
                )

            # --- EMA update on [128, 16] tiles (class = p*16 + g) ---------
            gs3 = gsum[:].rearrange("p (s g) -> p s g", s=3)
            n_t = gs3[:, 0, :]
            s_t = gs3[:, 1, :]
            q_t = gs3[:, 2, :]

            _t16_id = [0]

            def t16f(dtype=F32):
                _t16_id[0] += 1
                return sb.tile([P, NG], dtype, name=f"t16_{_t16_id[0]}")

            ns_t, rn_t, bm_t, bv_t = t16f(), t16f(), t16f(), t16f()
            nc.vector.tensor_scalar_max(out=ns_t[:], in0=n_t, scalar1=1.0)
            nc.vector.reciprocal(out=rn_t[:], in_=ns_t[:])
            nc.vector.tensor_tensor(out=bm_t[:], in0=s_t, in1=rn_t[:], op=OP.mult)
            qn_t, bm2_t = t16f(), t16f()
            nc.vector.tensor_tensor(out=qn_t[:], in0=q_t, in1=rn_t[:], op=OP.mult)
            nc.vector.tensor_tensor(out=bm2_t[:], in0=bm_t[:], in1=bm_t[:], op=OP.mult)
            nc.vector.tensor_tensor(
                out=bv_t[:], in0=qn_t[:], in1=bm2_t[:], op=OP.subtract
            )

            has_t = t16f(mybir.dt.uint8)
            nc.vector.tensor_scalar(
                out=has_t[:], in0=n_t, scalar1=0.0, scalar2=None, op0=OP.is_gt
            )

            d_t, em_t, ev_t = t16f(), t16f(), t16f()
            nc.vector.tensor_tensor(out=d_t[:], in0=bm_t[:], in1=m_t[:], op=OP.subtract)
            nc.vector.scalar_tensor_tensor(
                out=em_t[:], in0=d_t[:], scalar=EMA_DECAY, in1=m_t[:],
                op0=OP.mult, op1=OP.add,
            )
            nc.vector.tensor_tensor(
                out=d_t[:], in0=bv_t[:], in1=va_t[:], op=OP.subtract
            )
            nc.vector.scalar_tensor_tensor(
                out=ev_t[:], in0=d_t[:], scalar=EMA_DECAY, in1=va_t[:],
                op0=OP.mult, op1=OP.add,
            )

            cm_t, cv_t = t16f(), t16f()
            nc.vector.select(out=cm_t[:], mask=first_t[:], on_true=bm_t[:], on_false=em_t[:])
            nc.vector.select(out=cv_t[:], mask=first_t[:], on_true=bv_t[:], on_false=ev_t[:])
            nc.vector.tensor_scalar_max(out=cv_t[:], in0=cv_t[:], scalar1=EPS)

            nm_t, nv_t = t16f(), t16f()
            nc.vector.select(out=nm_t[:], mask=has_t[:], on_true=cm_t[:], on_false=m_t[:])
            nc.vector.select(out=nv_t[:], mask=has_t[:], on_true=cv_t[:], on_false=va_t[:])
            ni_t, ncnt_t = t16f(I32), t16f(I32)
            nc.vector.tensor_copy(out=ni_t[:], in_=n_t)
            nc.vector.tensor_tensor(out=ncnt_t[:], in0=c_t[:], in1=ni_t[:], op=OP.add)

            nc.sync.dma_start(
                out=new_mean[:].rearrange("(p c) -> p c", p=P), in_=nm_t[:]
            )
            nc.sync.dma_start(
                out=new_var[:].rearrange("(p c) -> p c", p=P), in_=nv_t[:]
            )
            nc.sync.dma_start(
                out=new_count[:].rearrange("(p c) -> p c", p=P), in_=ncnt_t[:]
            )

    nc.compile()
    return nc


def make_in_maps(logits, target, mean, var, count):
    """Shard the full inputs into per-core input maps."""
    logits = np.ascontiguousarray(np.asarray(logits, dtype=np.float32))
    target = np.asarray(target).astype(np.int32)
    mean = np.asarray(mean, dtype=np.float32)
    var = np.asarray(var, dtype=np.float32)
    count_i32 = np.asarray(count).astype(np.int32)

    in_maps = []
    for m in range(NCORES):
        rows = slice(m * BS, (m + 1) * BS)
        in_maps.append(
            {
                "logits": logits[rows].reshape(BS * K, 1),
                "target": target[rows],
                "mean": mean,
                "var": var,
                "count": count_i32,
            }
        )
    return in_maps


_NC_CACHE = None


def kernel(logits, target, mean, var, count):
    global _NC_CACHE
    if _NC_CACHE is None:
        _NC_CACHE = build_program()
    nc = _NC_CACHE

    in_maps = make_in_maps(logits, target, mean, var, count)
    res = run_bass_kernel_spmd(nc, in_maps, list(range(NCORES)))
    out = res.results[0]

    count_dtype = np.asarray(count).dtype
    return (
        out["new_mean"].reshape(K).astype(np.float32),
        out["new_var"].reshape(K).astype(np.float32),
        out["new_count"].reshape(K).astype(count_dtype),
    )
